# revision 3
# baseline (speedup 1.0000x reference)
"""Trainium2 Bass kernel for AdaBlock: binarized 3x3 conv (256->128) + bias +
PReLU + bias + scaled shortcut + pixel_unshuffle(2).

Strategy: pure data-parallel across 8 NeuronCores (2 images each). The conv is
an implicit GEMM: for each 512-pixel PSUM tile (4 rows x 128 cols), accumulate
18 matmuls (2 input-channel chunks x 3x3 taps) with K=128, M=128, N=512.
Weights are binarized on host (sign(w) * per-channel mean|w|, exactly the
reference math) and fed as bf16 [i, chunk, kh, kw, o] tiles; activations are
converted to bf16 on host. Halo rows/columns are zero-padded in SBUF (width
130) so every tap is a pure strided read. Epilogue per tile:
  r1 = relu(conv + b1); r2 = relu(-conv - b1); prelu = r1 - a*r2
  out = prelu + b2 + scale * x[:128]   (all fused on ScalarE/VectorE)
pixel_unshuffle is a free host-side reshape/transpose after gathering.
"""

import numpy as np
import ml_dtypes

B_FULL = 16
B_CORE = 2          # images per core (16 / 8 cores)
CIN = 256
COUT = 128
H = W = 128
RB = 16             # output rows per block
NBLK = H // RB      # 8 blocks per image
WP = W + 2          # padded width in SBUF
HALO = RB + 2       # input rows needed per block
N_CORES = 8

LAST_EXEC_NS = None
LAST_PROFILE = None

_cache = {}


def _build():
    import concourse.mybir as mybir
    import concourse.tile as tile
    from concourse import bacc

    fp32 = mybir.dt.float32
    bf16 = mybir.dt.bfloat16

    nc = bacc.Bacc("TRN2", target_bir_lowering=False, debug=False,
                   num_devices=N_CORES)

    x_ext = nc.dram_tensor("x", [B_CORE, CIN, H, W], bf16, kind="ExternalInput")
    w_ext = nc.dram_tensor("w", [128, 2, 3, 3, COUT], bf16, kind="ExternalInput")
    # packed per-channel params: cols = (b1, -b1, -alpha, b2, scale)
    p_ext = nc.dram_tensor("p", [COUT, 5], fp32, kind="ExternalInput")
    out_ext = nc.dram_tensor("out", [B_CORE, COUT, H, W], bf16,
                             kind="ExternalOutput")

    AF = mybir.ActivationFunctionType
    OP = mybir.AluOpType

    with tile.TileContext(nc) as tc:
        with tc.tile_pool(name="const", bufs=1) as cpool, \
             tc.tile_pool(name="xin", bufs=2) as xpool, \
             tc.tile_pool(name="outp", bufs=3) as opool, \
             tc.tile_pool(name="eps", bufs=4) as epool, \
             tc.tile_pool(name="psum", bufs=6, space="PSUM") as pspool:

            wt = cpool.tile([128, 2, 3, 3, COUT], bf16)
            nc.sync.dma_start(wt[:], w_ext[:])
            pt = cpool.tile([COUT, 5], fp32)
            nc.sync.dma_start(pt[:], p_ext[:])
            b1 = pt[:, 0:1]
            nb1 = pt[:, 1:2]
            na = pt[:, 2:3]
            b2 = pt[:, 3:4]
            sv = pt[:, 4:5]

            for b in range(B_CORE):
                for blk in range(NBLK):
                    r0 = blk * RB
                    xb = xpool.tile([128, 2, HALO, WP], bf16, tag="xb")
                    # zero the left/right padding columns
                    nc.gpsimd.memset(xb[:, :, :, 0], 0.0)
                    nc.gpsimd.memset(xb[:, :, :, WP - 1], 0.0)
                    lo = max(r0 - 1, 0)
                    hi = min(r0 + RB + 1, H)
                    off = lo - (r0 - 1)
                    if off:
                        nc.gpsimd.memset(xb[:, :, 0, :], 0.0)
                    if hi - lo + off < HALO:
                        nc.gpsimd.memset(xb[:, :, HALO - 1, :], 0.0)
                    for c in range(2):
                        nc.sync.dma_start(
                            xb[:, c, off:off + (hi - lo), 1:1 + W],
                            x_ext[b, c * 128:(c + 1) * 128, lo:hi, :])

                    ob = opool.tile([COUT, RB, W], bf16, tag="ob")
                    sc = epool.tile([COUT, RB, W], fp32, tag="sc")
                    # shortcut = scale * x[:, :128] + b2 (valid rows of chunk 0)
                    nc.vector.tensor_scalar(
                        sc[:], xb[:, 0, 1:1 + RB, 1:1 + W], sv, b2,
                        OP.mult, OP.add)

                    for g in range(RB // 4):
                        ps = pspool.tile([COUT, 512], fp32, tag="ps")
                        idx = 0
                        for c in range(2):
                            for kh in range(3):
                                for kw in range(3):
                                    nc.tensor.matmul(
                                        ps[:],
                                        wt[:, c, kh, kw, :],
                                        xb[:, c, 4 * g + kh:4 * g + kh + 4,
                                           kw:kw + W],
                                        start=(idx == 0), stop=(idx == 17))
                                    idx += 1
                        r1 = epool.tile([COUT, 512], fp32, tag="r1")
                        nc.scalar.activation(r1[:], ps[:], AF.Relu,
                                             bias=b1, scale=1.0)
                        r2 = epool.tile([COUT, 512], fp32, tag="r2")
                        nc.scalar.activation(r2[:], ps[:], AF.Relu,
                                             bias=nb1, scale=-1.0)
                        m = epool.tile([COUT, 512], fp32, tag="m")
                        nc.vector.tensor_scalar(m[:], r2[:], na, None, OP.mult)
                        t = epool.tile([COUT, 512], fp32, tag="t")
                        nc.vector.tensor_tensor(t[:], r1[:], m[:], OP.add)
                        nc.vector.tensor_tensor(
                            ob[:, 4 * g:4 * g + 4, :],
                            t[:], sc[:, 4 * g:4 * g + 4, :],
                            OP.add)

                    nc.sync.dma_start(out_ext[b, :, r0:r0 + RB, :], ob[:])

    nc.compile()
    return nc


def kernel(x, conv_w, move1_b, prelu_w, move2_b, scale, _trace=False):
    global LAST_EXEC_NS, LAST_PROFILE
    assert x.shape == (B_FULL, CIN, H, W), x.shape

    # --- host-side weight binarization (exact reference math, fp32) ---
    w32 = conv_w.astype(np.float32)
    alpha = np.mean(np.abs(w32), axis=(1, 2, 3), keepdims=True)   # [O,1,1,1]
    wb = np.sign(w32) * alpha                                     # [O,I,3,3]
    # device layout: [i_in_chunk, chunk, kh, kw, o]
    wl = wb.reshape(COUT, 2, 128, 3, 3).transpose(2, 1, 3, 4, 0)
    wl = np.ascontiguousarray(wl).astype(ml_dtypes.bfloat16)

    params = np.stack([
        move1_b.astype(np.float32),
        -move1_b.astype(np.float32),
        -prelu_w.astype(np.float32),
        move2_b.astype(np.float32),
        np.full((COUT,), float(scale[0]), np.float32),
    ], axis=1)
    params = np.ascontiguousarray(params)

    xb16 = x.astype(ml_dtypes.bfloat16)

    if "nc" not in _cache:
        _cache["nc"] = _build()
    nc = _cache["nc"]

    in_maps = []
    for i in range(N_CORES):
        in_maps.append({
            "x": np.ascontiguousarray(xb16[i * B_CORE:(i + 1) * B_CORE]),
            "w": wl,
            "p": params,
        })

    from concourse.bass_utils import run_bass_kernel_spmd
    res = run_bass_kernel_spmd(nc, in_maps, core_ids=list(range(N_CORES)),
                               trace=_trace)
    LAST_EXEC_NS = res.exec_time_ns
    LAST_PROFILE = res
    out = np.concatenate([res.results[i]["out"] for i in range(N_CORES)],
                         axis=0).astype(np.float32)   # [16,128,128,128]

    # pixel_unshuffle2: [B,C,H,W] -> [B,C*4,H/2,W/2]
    B, C, HH, WW = out.shape
    out = out.reshape(B, C, HH // 2, 2, WW // 2, 2)
    out = out.transpose(0, 1, 3, 5, 2, 4)
    return np.ascontiguousarray(out.reshape(B, C * 4, HH // 2, WW // 2))


# revision 6
# speedup vs baseline: 1.1669x; 1.1669x over previous
"""Trainium2 Bass kernel for AdaBlock: binarized 3x3 conv (256->128) + bias +
PReLU + bias + scaled shortcut + pixel_unshuffle(2).

Strategy: pure data-parallel across 8 NeuronCores (2 images each). The conv is
an implicit GEMM: for each 512-pixel PSUM tile (4 rows x 128 cols), accumulate
18 matmuls (2 input-channel chunks x 3x3 taps) with K=128, M=128, N=512.
Weights are binarized on host (sign(w) * per-channel mean|w|, exactly the
reference math) and fed as bf16 [i, chunk, kh, kw, o] tiles; activations are
converted to bf16 on host. Halo rows/columns are zero-padded in SBUF (width
130) so every tap is a pure strided read. Epilogue per tile:
  r1 = relu(conv + b1); r2 = relu(-conv - b1); prelu = r1 - a*r2
  out = prelu + b2 + scale * x[:128]   (all fused on ScalarE/VectorE)
pixel_unshuffle is a free host-side reshape/transpose after gathering.
"""

import numpy as np
import ml_dtypes

B_FULL = 16
B_CORE = 2          # images per core (16 / 8 cores)
CIN = 256
COUT = 128
H = W = 128
RB = 16             # output rows per block
NBLK = H // RB      # 8 blocks per image
WP = W + 2          # padded width in SBUF
HALO = RB + 2       # input rows needed per block
N_CORES = 8

LAST_EXEC_NS = None
LAST_PROFILE = None

_cache = {}


def _build():
    import concourse.mybir as mybir
    import concourse.tile as tile
    from concourse import bacc

    fp32 = mybir.dt.float32
    bf16 = mybir.dt.bfloat16

    nc = bacc.Bacc("TRN2", target_bir_lowering=False, debug=False,
                   num_devices=N_CORES)

    x_ext = nc.dram_tensor("x", [B_CORE, CIN, H, W], bf16, kind="ExternalInput")
    w_ext = nc.dram_tensor("w", [128, 2, 3, 3, COUT], bf16, kind="ExternalInput")
    # packed per-channel params: cols = (b1, -b1, -alpha, b2, scale)
    p_ext = nc.dram_tensor("p", [COUT, 5], fp32, kind="ExternalInput")
    out_ext = nc.dram_tensor("out", [B_CORE, COUT, H, W], bf16,
                             kind="ExternalOutput")

    AF = mybir.ActivationFunctionType
    OP = mybir.AluOpType

    with tile.TileContext(nc) as tc:
        with tc.tile_pool(name="const", bufs=1) as cpool, \
             tc.tile_pool(name="xin", bufs=2) as xpool, \
             tc.tile_pool(name="outp", bufs=3) as opool, \
             tc.tile_pool(name="eps", bufs=4) as epool, \
             tc.tile_pool(name="psum", bufs=2, space="PSUM") as pspool:

            wt = cpool.tile([128, 2, 3, 3, COUT], bf16)
            nc.sync.dma_start(wt[:], w_ext[:])
            pt = cpool.tile([COUT, 5], fp32)
            nc.sync.dma_start(pt[:], p_ext[:])
            b1 = pt[:, 0:1]
            nb1 = pt[:, 1:2]
            na = pt[:, 2:3]
            b2 = pt[:, 3:4]
            sv = pt[:, 4:5]

            for b in range(B_CORE):
                for blk in range(NBLK):
                    r0 = blk * RB
                    xb = xpool.tile([128, 2, HALO, WP], bf16, tag="xb")
                    # zero the left/right padding columns
                    nc.gpsimd.memset(xb[:, :, :, 0], 0.0)
                    nc.gpsimd.memset(xb[:, :, :, WP - 1], 0.0)
                    lo = max(r0 - 1, 0)
                    hi = min(r0 + RB + 1, H)
                    off = lo - (r0 - 1)
                    if off:
                        nc.gpsimd.memset(xb[:, :, 0, :], 0.0)
                    if hi - lo + off < HALO:
                        nc.gpsimd.memset(xb[:, :, HALO - 1, :], 0.0)
                    for c in range(2):
                        nc.sync.dma_start(
                            xb[:, c, off:off + (hi - lo), 1:1 + W],
                            x_ext[b, c * 128:(c + 1) * 128, lo:hi, :])

                    ob = opool.tile([COUT, RB, W], bf16, tag="ob")
                    sc = epool.tile([COUT, RB, W], fp32, tag="sc")
                    # shortcut = scale * x[:, :128] + b2 (valid rows of chunk 0)
                    nc.vector.tensor_scalar(
                        sc[:], xb[:, 0, 1:1 + RB, 1:1 + W], sv, b2,
                        OP.mult, OP.add)

                    NG = RB // 4
                    pss = [pspool.tile([COUT, 512], fp32, tag=f"ps{g}",
                                       name=f"ps{g}")
                           for g in range(NG)]
                    idx = 0
                    for c in range(2):
                        for kh in range(3):
                            for kw in range(3):
                                for g in range(NG):
                                    nc.tensor.matmul(
                                        pss[g][:],
                                        wt[:, c, kh, kw, :],
                                        xb[:, c, 4 * g + kh:4 * g + kh + 4,
                                           kw:kw + W],
                                        start=(idx == 0), stop=(idx == 17))
                                idx += 1
                    for g in range(NG):
                        ps = pss[g]
                        r1 = epool.tile([COUT, 512], fp32, tag="r1")
                        nc.scalar.activation(r1[:], ps[:], AF.Relu,
                                             bias=b1, scale=1.0)
                        r2 = epool.tile([COUT, 512], fp32, tag="r2")
                        nc.scalar.activation(r2[:], ps[:], AF.Relu,
                                             bias=nb1, scale=-1.0)
                        m = epool.tile([COUT, 512], fp32, tag="m")
                        nc.vector.tensor_scalar(m[:], r2[:], na, None, OP.mult)
                        t = epool.tile([COUT, 512], fp32, tag="t")
                        nc.vector.tensor_tensor(t[:], r1[:], m[:], OP.add)
                        nc.vector.tensor_tensor(
                            ob[:, 4 * g:4 * g + 4, :],
                            t[:], sc[:, 4 * g:4 * g + 4, :],
                            OP.add)

                    nc.sync.dma_start(out_ext[b, :, r0:r0 + RB, :], ob[:])

    nc.compile()
    return nc


def kernel(x, conv_w, move1_b, prelu_w, move2_b, scale, _trace=False):
    global LAST_EXEC_NS, LAST_PROFILE
    assert x.shape == (B_FULL, CIN, H, W), x.shape

    # --- host-side weight binarization (exact reference math, fp32) ---
    w32 = conv_w.astype(np.float32)
    alpha = np.mean(np.abs(w32), axis=(1, 2, 3), keepdims=True)   # [O,1,1,1]
    wb = np.sign(w32) * alpha                                     # [O,I,3,3]
    # device layout: [i_in_chunk, chunk, kh, kw, o]
    wl = wb.reshape(COUT, 2, 128, 3, 3).transpose(2, 1, 3, 4, 0)
    wl = np.ascontiguousarray(wl).astype(ml_dtypes.bfloat16)

    params = np.stack([
        move1_b.astype(np.float32),
        -move1_b.astype(np.float32),
        -prelu_w.astype(np.float32),
        move2_b.astype(np.float32),
        np.full((COUT,), float(scale[0]), np.float32),
    ], axis=1)
    params = np.ascontiguousarray(params)

    xb16 = x.astype(ml_dtypes.bfloat16)

    if "nc" not in _cache:
        _cache["nc"] = _build()
    nc = _cache["nc"]

    in_maps = []
    for i in range(N_CORES):
        in_maps.append({
            "x": np.ascontiguousarray(xb16[i * B_CORE:(i + 1) * B_CORE]),
            "w": wl,
            "p": params,
        })

    from concourse.bass_utils import run_bass_kernel_spmd
    res = run_bass_kernel_spmd(nc, in_maps, core_ids=list(range(N_CORES)),
                               trace=_trace)
    LAST_EXEC_NS = res.exec_time_ns
    LAST_PROFILE = res
    out = np.concatenate([res.results[i]["out"] for i in range(N_CORES)],
                         axis=0).astype(np.float32)   # [16,128,128,128]

    # pixel_unshuffle2: [B,C,H,W] -> [B,C*4,H/2,W/2]
    B, C, HH, WW = out.shape
    out = out.reshape(B, C, HH // 2, 2, WW // 2, 2)
    out = out.transpose(0, 1, 3, 5, 2, 4)
    return np.ascontiguousarray(out.reshape(B, C * 4, HH // 2, WW // 2))


# revision 8
# speedup vs baseline: 1.1902x; 1.0200x over previous
"""Trainium2 Bass kernel for AdaBlock: binarized 3x3 conv (256->128) + bias +
PReLU + bias + scaled shortcut + pixel_unshuffle(2).

Strategy: pure data-parallel across 8 NeuronCores (2 images each). The conv is
an implicit GEMM: for each 512-pixel PSUM tile (4 rows x 128 cols), accumulate
18 matmuls (2 input-channel chunks x 3x3 taps) with K=128, M=128, N=512.
Weights are binarized on host (sign(w) * per-channel mean|w|, exactly the
reference math) and fed as bf16 [i, chunk, kh, kw, o] tiles; activations are
converted to bf16 on host. Halo rows/columns are zero-padded in SBUF (width
130) so every tap is a pure strided read. Epilogue per tile:
  r1 = relu(conv + b1); r2 = relu(-conv - b1); prelu = r1 - a*r2
  out = prelu + b2 + scale * x[:128]   (all fused on ScalarE/VectorE)
pixel_unshuffle is a free host-side reshape/transpose after gathering.
"""

import numpy as np
import ml_dtypes

B_FULL = 16
B_CORE = 2          # images per core (16 / 8 cores)
CIN = 256
COUT = 128
H = W = 128
RB = 16             # output rows per block
NBLK = H // RB      # 8 blocks per image
WP = W + 2          # padded width in SBUF
HALO = RB + 2       # input rows needed per block
N_CORES = 8

LAST_EXEC_NS = None
LAST_PROFILE = None

_cache = {}


def _build():
    import concourse.mybir as mybir
    import concourse.tile as tile
    from concourse import bacc

    fp32 = mybir.dt.float32
    bf16 = mybir.dt.bfloat16

    nc = bacc.Bacc("TRN2", target_bir_lowering=False, debug=False,
                   num_devices=N_CORES)

    x_ext = nc.dram_tensor("x", [B_CORE, CIN, H, W], bf16, kind="ExternalInput")
    w_ext = nc.dram_tensor("w", [128, 2, 3, 3, COUT], bf16, kind="ExternalInput")
    # packed per-channel params: cols = (b1, -b1, -alpha, b2, scale)
    p_ext = nc.dram_tensor("p", [COUT, 5], fp32, kind="ExternalInput")
    out_ext = nc.dram_tensor("out", [B_CORE, COUT, H, W], bf16,
                             kind="ExternalOutput")

    AF = mybir.ActivationFunctionType
    OP = mybir.AluOpType

    with tile.TileContext(nc) as tc:
        with tc.tile_pool(name="const", bufs=1) as cpool, \
             tc.tile_pool(name="xin", bufs=2) as xpool, \
             tc.tile_pool(name="outp", bufs=3) as opool, \
             tc.tile_pool(name="eps", bufs=4) as epool, \
             tc.tile_pool(name="psum", bufs=2, space="PSUM") as pspool:

            wt = cpool.tile([128, 2, 3, 3, COUT], bf16)
            nc.sync.dma_start(wt[:], w_ext[:])
            pt = cpool.tile([COUT, 5], fp32)
            nc.sync.dma_start(pt[:], p_ext[:])
            b1 = pt[:, 0:1]
            nb1 = pt[:, 1:2]
            na = pt[:, 2:3]
            b2 = pt[:, 3:4]
            sv = pt[:, 4:5]

            TAPS = [(c, kh, kw) for c in range(2) for kh in range(3)
                    for kw in range(3)]

            def emit_block(b, r0, nrows, gsplit):
                halo = nrows + 2
                xb = xpool.tile([128, 2, halo, WP], bf16, tag="xb",
                                name="xb")
                # zero the left/right padding columns
                nc.gpsimd.memset(xb[:, :, :, 0], 0.0)
                nc.gpsimd.memset(xb[:, :, :, WP - 1], 0.0)
                lo = max(r0 - 1, 0)
                hi = min(r0 + nrows + 1, H)
                off = lo - (r0 - 1)
                if off:
                    nc.gpsimd.memset(xb[:, :, 0, :], 0.0)
                if hi - lo + off < halo:
                    nc.gpsimd.memset(xb[:, :, halo - 1, :], 0.0)
                for c in range(2):
                    nc.sync.dma_start(
                        xb[:, c, off:off + (hi - lo), 1:1 + W],
                        x_ext[b, c * 128:(c + 1) * 128, lo:hi, :])

                ob = opool.tile([COUT, nrows, W], bf16, tag="ob",
                                name="ob")
                sc = epool.tile([COUT, nrows, W], fp32, tag="sc",
                                name="sc")
                # shortcut = scale * x[:, :128] + b2 (valid rows of chunk 0)
                nc.scalar.activation(sc[:], xb[:, 0, 1:1 + nrows, 1:1 + W],
                                     AF.Identity, bias=b2, scale=sv)

                NG = nrows // 4

                def epilogue(g, ps):
                    r1 = epool.tile([COUT, 512], fp32, tag="r1", name="r1")
                    nc.scalar.activation(r1[:], ps[:], AF.Relu,
                                         bias=b1, scale=1.0)
                    r2 = epool.tile([COUT, 512], fp32, tag="r2", name="r2")
                    nc.scalar.activation(r2[:], ps[:], AF.Relu,
                                         bias=nb1, scale=-1.0)
                    m = epool.tile([COUT, 512], fp32, tag="m", name="m")
                    nc.vector.tensor_scalar(m[:], r2[:], na, None, OP.mult)
                    t = epool.tile([COUT, 512], fp32, tag="t", name="t")
                    nc.vector.tensor_tensor(t[:], r1[:], m[:], OP.add)
                    nc.vector.tensor_tensor(
                        ob[:, 4 * g:4 * g + 4, :],
                        t[:], sc[:, 4 * g:4 * g + 4, :],
                        OP.add)

                for gs in range(0, NG, gsplit):
                    gset = list(range(gs, min(gs + gsplit, NG)))
                    pss = {}
                    for idx, (c, kh, kw) in enumerate(TAPS):
                        for g in gset:
                            if idx == 0:
                                pss[g] = pspool.tile(
                                    [COUT, 512], fp32,
                                    tag=f"ps{g}", name=f"ps{g}")
                            nc.tensor.matmul(
                                pss[g][:],
                                wt[:, c, kh, kw, :],
                                xb[:, c, 4 * g + kh:4 * g + kh + 4,
                                   kw:kw + W],
                                start=(idx == 0), stop=(idx == 17))
                    for g in gset:
                        epilogue(g, pss[g])

                nc.sync.dma_start(out_ext[b, :, r0:r0 + nrows, :], ob[:])

            blocks = []
            for b in range(B_CORE):
                for blk in range(NBLK):
                    blocks.append((b, blk * RB, RB))
            # split the first block so PE starts after a small DMA
            blocks[0] = (0, 0, 4)
            blocks.insert(1, (0, 4, RB - 4))

            for i, (b, r0, nrows) in enumerate(blocks):
                last = i == len(blocks) - 1
                # last block: pair-split so its epilogue overlaps its matmuls
                emit_block(b, r0, nrows, 2 if last else nrows // 4)

    nc.compile()
    return nc


def kernel(x, conv_w, move1_b, prelu_w, move2_b, scale, _trace=False):
    global LAST_EXEC_NS, LAST_PROFILE
    assert x.shape == (B_FULL, CIN, H, W), x.shape

    # --- host-side weight binarization (exact reference math, fp32) ---
    w32 = conv_w.astype(np.float32)
    alpha = np.mean(np.abs(w32), axis=(1, 2, 3), keepdims=True)   # [O,1,1,1]
    wb = np.sign(w32) * alpha                                     # [O,I,3,3]
    # device layout: [i_in_chunk, chunk, kh, kw, o]
    wl = wb.reshape(COUT, 2, 128, 3, 3).transpose(2, 1, 3, 4, 0)
    wl = np.ascontiguousarray(wl).astype(ml_dtypes.bfloat16)

    params = np.stack([
        move1_b.astype(np.float32),
        -move1_b.astype(np.float32),
        -prelu_w.astype(np.float32),
        move2_b.astype(np.float32),
        np.full((COUT,), float(scale[0]), np.float32),
    ], axis=1)
    params = np.ascontiguousarray(params)

    xb16 = x.astype(ml_dtypes.bfloat16)

    if "nc" not in _cache:
        _cache["nc"] = _build()
    nc = _cache["nc"]

    in_maps = []
    for i in range(N_CORES):
        in_maps.append({
            "x": np.ascontiguousarray(xb16[i * B_CORE:(i + 1) * B_CORE]),
            "w": wl,
            "p": params,
        })

    from concourse.bass_utils import run_bass_kernel_spmd
    res = run_bass_kernel_spmd(nc, in_maps, core_ids=list(range(N_CORES)),
                               trace=_trace)
    LAST_EXEC_NS = res.exec_time_ns
    LAST_PROFILE = res
    out = np.concatenate([res.results[i]["out"] for i in range(N_CORES)],
                         axis=0).astype(np.float32)   # [16,128,128,128]

    # pixel_unshuffle2: [B,C,H,W] -> [B,C*4,H/2,W/2]
    B, C, HH, WW = out.shape
    out = out.reshape(B, C, HH // 2, 2, WW // 2, 2)
    out = out.transpose(0, 1, 3, 5, 2, 4)
    return np.ascontiguousarray(out.reshape(B, C * 4, HH // 2, WW // 2))
